# revision 13
# baseline (speedup 1.0000x reference)
"""BoxFilter (9x9 unnormalized box sum, zero-padded borders) on 8 trn2 cores.

Full input: image [8, 32, 512, 512] f32, batch-sharded: core b handles
image[b]. Device I/O is bf16 (host converts): halves HBM traffic in the
memory-bound regime. Per channel slice X [512, 512]:

  pass A (H) on PE: Y[i-block, w] = sum_j Band[j, i] X[j, w] using the three
    Toeplitz blocks of the 9-band matrix (diagonal + two corners) as bf16
    stationaries -- 10 matmuls per slice, f32 PSUM accumulation (exact sums
    of bf16 inputs).
  PSUM eviction on the Activation engine (f32 -> fp16 zero-padded SBUF
    rows; fp16 costs the same as bf16 here and is 8x more precise),
    keeping the DVE free.
  pass B (W) on DVE: one tensor_tensor_scan per slice over the
    concatenated zero-padded rows (state = (Y[j+9] + state) - Y[j],
    telescoping to the 9-tap box; ~2.2 cyc/step, recurrence-bound --
    measured faster than every alternative, see W_MODE comment).
  stores issued from the Activation engine so the SP queue only carries
    loads (a store waiting on compute otherwise blocks the next load).

One DMA loads all 4 h-chunks of a slice, one DMA stores all 4 h-blocks.
Errors: bf16 input quantization + fp16 H-sums + one bf16 output rounding,
~3e-3 of output scale vs the 2e-2 gate.
"""

import numpy as np
import ml_dtypes

import concourse.bass as bass
import concourse.mybir as mybir
import concourse.tile as tile
from concourse import bacc, bass_utils

RADIUS = 4
H = W = 512
P = 128  # partitions / chunk size
NCHUNK = H // P  # 4
N_CORES = 8
NCH = 32  # channels per core (batch dim sharded across cores)

YPW = 9 + W + 9  # scan row: 9 lead + data + 9 tail zeros
NW = NCHUNK * YPW  # 2120
OW = NW - 9  # scan output width; boxW[w] lands at col YPW*d + 4 + w

W_MODE = "scan"  # "scan" | "pair" | "tree" | "mix"
# Alternatives all measured SLOWER than the plain DVE scan (149-153us):
# pair (231us): odd-fill throttles on Pool (~7us/ch strided). tree (174+):
# ties the scan on DVE; any Pool op feeding DVE regresses further. mix
# (316us): even Pool-ONLY tree channels (disjoint from DVE) run ~40us/ch
# in-pipeline vs 14us predicted from isolated probes -- GPSIMD is not a
# usable second vector lane here.
MIX_MOD = 4  # every MIX_MOD-th channel takes the Pool tree (mix mode)
TREE_ENG = "pvvv"  # engine per tree level: p=Pool, v=DVE

BF16 = ml_dtypes.bfloat16


def band_constant(scale: float = 1.0) -> np.ndarray:
    """[128, 384] bf16: the three Toeplitz blocks of the 9-band matrix --
    diagonal block | lower corner (prev chunk) | upper corner (next chunk)."""
    j = np.arange(P)[:, None]
    i = np.arange(P)[None, :]
    b0 = (np.abs(i - j) <= RADIUS).astype(np.float32)
    bm = (np.abs(128 + i - j) <= RADIUS).astype(np.float32)
    bp = (np.abs(i - j - 128) <= RADIUS).astype(np.float32)
    return (scale * np.concatenate([b0, bm, bp], axis=1)).astype(BF16)


def _emit_pass(nc, pools, band_r, x_ap, y_ap, nch):
    """Emit the full boxfilter for one [nch, H, W] bf16 tensor pair."""
    f32 = mybir.dt.float32
    bf16 = mybir.dt.bfloat16
    x_pool, yp_tiles, pt_tiles, o_pool, psA = pools
    engmap = {"v": nc.vector, "p": nc.gpsimd}
    for c in range(nch):
        # one DMA for all 4 h-chunks: xbig[p, (t, w)] <- x[c, 128t + p, w]
        xbig = x_pool.tile([P, NCHUNK * W], bf16, tag="x")
        nc.sync.dma_start(
            xbig[:].rearrange("p (t w) -> p t w", t=NCHUNK),
            x_ap[c].rearrange("(t p) w -> p t w", p=P),
        )
        xt = [xbig[:, W * t : W * t + W] for t in range(NCHUNK)]

        # all 4 h-blocks in one 4-bank PSUM tile
        y_ps = psA.tile([P, NCHUNK * W], f32)
        for d in range(NCHUNK):  # h i-block
            mms = [(0, d)]
            if d >= 1:
                mms.append((1, d - 1))
            if d <= NCHUNK - 2:
                mms.append((2, d + 1))
            for k, (m, t) in enumerate(mms):
                nc.tensor.matmul(
                    y_ps[:, W * d : W * d + W],
                    lhsT=band_r[m],
                    rhs=xt[t],
                    start=(k == 0),
                    stop=(k == len(mms) - 1),
                )
        # evacuate PSUM on the Activation engine into the zero-padded scan
        # rows (only data columns written; pads stay zero forever)
        yp = yp_tiles[c % len(yp_tiles)]
        nc.scalar.copy(
            yp[:].rearrange("p (d u) -> p d u", d=NCHUNK)[:, :, 9 : 9 + W],
            y_ps[:].rearrange("p (d u) -> p d u", d=NCHUNK),
        )
        obig = o_pool.tile([P, NW], bf16, tag="o")
        if W_MODE == "mix":
            add = mybir.AluOpType.add
            if c % MIX_MOD == MIX_MOD - 1:
                # Pool-only shift-add tree (flat 2D fp16): feeds nothing on
                # the DVE side, so its throughput adds in parallel
                f16 = mybir.dt.float16
                e = nc.gpsimd
                t1 = x_pool.tile([P, NW], f16, tag="t1")
                e.tensor_tensor(t1[:, 0 : NW - 6], yp[:, 5 : NW - 1], yp[:, 6:NW], add)
                t2 = x_pool.tile([P, NW], f16, tag="t2")
                e.tensor_tensor(t2[:, 0 : NW - 8], t1[:, 0 : NW - 8], t1[:, 2 : NW - 6], add)
                t4 = x_pool.tile([P, NW], f16, tag="t4")
                e.tensor_tensor(t4[:, 0 : NW - 12], t2[:, 0 : NW - 12], t2[:, 4 : NW - 8], add)
                e.tensor_tensor(obig[:, 4 : NW - 9], t4[:, 0 : NW - 13], yp[:, 13:NW], add)
            else:
                nc.vector.tensor_tensor_scan(
                    obig[:, 0:OW], yp[:, 9:NW], yp[:, 0:OW],
                    0.0, add, mybir.AluOpType.subtract,
                )
        elif W_MODE == "pair":
            # stride-2 telescoped scan: halves the DVE recurrence steps.
            #   pt[j] = yp[j-2] + yp[j-1]            (DVE, flat bf16)
            #   even boxes: state(k) += pt[2k+10] - pt[2k+1] -> obig[2k]
            #   odd boxes (Pool, downstream of the scan; only the store
            #   waits on them): D[k] = yp[2k+10] - yp[2k+1],
            #   obig[2k+1] = obig[2k] + D[k]
            add, sub = mybir.AluOpType.add, mybir.AluOpType.subtract
            pt = pt_tiles[c % len(pt_tiles)]
            nc.vector.tensor_tensor(
                pt[:, 2:NW], yp[:, 0 : NW - 2], yp[:, 1 : NW - 1], add
            )
            ptE = pt[:].rearrange("p (k two) -> p k two", two=2)[:, :, 0]
            ptO = pt[:].rearrange("p (k two) -> p k two", two=2)[:, :, 1]
            obE = obig[:].rearrange("p (k two) -> p k two", two=2)[:, :, 0]
            obO = obig[:].rearrange("p (k two) -> p k two", two=2)[:, :, 1]
            nc.vector.tensor_tensor_scan(
                obE[:, 0:1055], ptE[:, 5:1060], ptO[:, 0:1055], 0.0, add, sub
            )
            ypE = yp[:].rearrange("p (k two) -> p k two", two=2)[:, :, 0]
            ypO = yp[:].rearrange("p (k two) -> p k two", two=2)[:, :, 1]
            dt_ = o_pool.tile([P, 1056], mybir.dt.float16, tag="d")
            nc.gpsimd.tensor_tensor(
                dt_[:, 0:1055], ypE[:, 5:1060], ypO[:, 0:1055], sub
            )
            nc.gpsimd.tensor_tensor(
                obO[:, 0:1055], obE[:, 0:1055], dt_[:, 0:1055], add
            )
        elif W_MODE == "scan":
            # one scan emits the 9-tap running box for all 4 blocks (18
            # zeros sit between blocks, so the telescoped sum never crosses)
            nc.vector.tensor_tensor_scan(
                obig[:, 0:OW],
                yp[:, 9:NW],
                yp[:, 0:OW],
                0.0,
                mybir.AluOpType.add,
                mybir.AluOpType.subtract,
            )
        else:
            # flat shift-add tree: t1=2sum, t2=4sum, t4=8sum, out=t4+y>>8;
            # flat 2D ops across the padded blocks (cross-block positions
            # only combine pad zeros and land in pad columns)
            e1, e2, e3, e4 = (engmap[ch] for ch in TREE_ENG)
            add = mybir.AluOpType.add
            t1 = x_pool.tile([P, NW], bf16, tag="t1")
            e1.tensor_tensor(t1[:, 0 : NW - 6], yp[:, 5 : NW - 1], yp[:, 6:NW], add)
            t2 = x_pool.tile([P, NW], bf16, tag="t2")
            e2.tensor_tensor(t2[:, 0 : NW - 8], t1[:, 0 : NW - 8], t1[:, 2 : NW - 6], add)
            t4 = x_pool.tile([P, NW], bf16, tag="t4")
            e3.tensor_tensor(t4[:, 0 : NW - 12], t2[:, 0 : NW - 12], t2[:, 4 : NW - 8], add)
            e4.tensor_tensor(obig[:, 4 : NW - 9], t4[:, 0 : NW - 13], yp[:, 13:NW], add)
        # store from the Activation engine (keeps the SP queue loads-only);
        # one DMA for all 4 h-blocks: y[c, 128d + p, w] <- obig[p, YPW*d+4+w]
        nc.scalar.dma_start(
            y_ap[c].rearrange("(d p) w -> p d w", p=P),
            obig[:].rearrange("p (d u) -> p d u", d=NCHUNK)[:, :, 4 : 4 + W],
        )


def make_pools(nc, tc, stack_pools):
    """Enter the SBUF/PSUM pools and pre-zero the two persistent scan rows."""
    bf16 = mybir.dt.bfloat16
    x_pool = stack_pools.enter_context(tc.tile_pool(name="xin", bufs=6))
    yt_pool = stack_pools.enter_context(tc.tile_pool(name="yt", bufs=1))
    o_pool = stack_pools.enter_context(tc.tile_pool(name="osb", bufs=6))
    psA = stack_pools.enter_context(tc.tile_pool(name="psA", bufs=2, space="PSUM"))
    f16 = mybir.dt.float16
    yp_tiles = []
    for i in range(2):
        t = yt_pool.tile([P, NW], f16, tag=f"yp{i}", name=f"yp{i}")
        nc.vector.memset(t[:], 0.0)
        yp_tiles.append(t)
    pt_tiles = []
    for i in range(2):
        t = yt_pool.tile([P, NW], f16, tag=f"pt{i}", name=f"pt{i}")
        nc.vector.memset(t[:], 0.0)
        pt_tiles.append(t)
    return (x_pool, yp_tiles, pt_tiles, o_pool, psA)


def build_nc(nch: int = NCH):
    from contextlib import ExitStack

    bf16 = mybir.dt.bfloat16
    nc = bacc.Bacc("TRN2", target_bir_lowering=False, debug=False)
    x = nc.dram_tensor("x", [nch, H, W], bf16, kind="ExternalInput").ap()
    band_d = nc.dram_tensor("band", [P, 3 * P], bf16, kind="ExternalInput").ap()
    y = nc.dram_tensor("y", [nch, H, W], bf16, kind="ExternalOutput").ap()

    with tile.TileContext(nc) as tc:
        with ExitStack() as stack:
            const_pool = stack.enter_context(tc.tile_pool(name="const", bufs=1))
            band_sb = const_pool.tile([P, 3 * P], bf16)
            nc.sync.dma_start(band_sb[:], band_d[:])
            band_r = [band_sb[:, P * m : P * m + P] for m in range(3)]
            pools = make_pools(nc, tc, stack)
            _emit_pass(nc, pools, band_r, x, y, nch)

    nc.compile()
    return nc


def kernel(image) -> np.ndarray:
    image = np.asarray(image)
    assert image.shape == (N_CORES, NCH, H, W), image.shape
    image_bf = image.astype(BF16)
    nc = build_nc(NCH)
    band = band_constant()
    in_maps = [{"x": image_bf[b], "band": band} for b in range(N_CORES)]
    res = bass_utils.run_bass_kernel_spmd(nc, in_maps, core_ids=list(range(N_CORES)))
    return np.stack([r["y"].astype(np.float32) for r in res.results], axis=0)


if __name__ == "__main__":
    img = np.random.rand(N_CORES, NCH, H, W).astype(np.float32)
    out = kernel(img)
    print(out.shape, out.dtype)
